# revision 25
# baseline (speedup 1.0000x reference)
"""Trainium2 Bass kernel for EquivariantSubSampling.

The reference module reduces to a per-batch gather (verified numerically):
with (oh, ow, r) = p[b] (each in {0,1}), ic = 2*oc + r:
    r=0: out[b, oc, a, c] = x[b, ic, oh + 2a, ow + 2c]
    r=1: out[b, oc, a, c] = x[b, ic, oh + 2*((32-c) % 32), ow + 2a]

Strategy: pure data parallel over the batch dim (16 batches / 8 cores = 2
per core).  Raw bacc program (no Tile framework — avoids its multi-us
preamble/teardown barriers).  Per batch, on device:
  - p[b] is DMAed to SBUF and read into engine registers
  - the needed input rows x[b, r::2, oh::2, :] are loaded with
    register-offset (dynamic) DMAs, split across the two HWDGE rings
    (sync + scalar engines) to parallelize descriptor generation
  - both gather variants are computed unconditionally into one tile
    (V[:, 0] = r0-variant, V[:, 1] = r1-variant), split across the
    vector and scalar engines; the output DMA then reads V[:, ds(r, 1)]
    (dynamic SBUF offset) — a branchless select
  - gpsimd clears the semaphores at the end so the NEFF is re-executable
"""

import numpy as np

B, C, H, W = 16, 256, 64, 64
NCORES = 8
BPC = B // NCORES           # batches per core
OC, OHW = 128, 32           # output channels, output spatial

_COMPILED = {}


def build_nc(enable_asserts=False, single_packet=True, no_gpsimd_drain=True):
    from contextlib import ExitStack

    import concourse.bacc as bacc
    import concourse.bass as bass
    import concourse.mybir as mybir

    ds = bass.ds
    f32 = mybir.dt.float32
    i32 = mybir.dt.int32
    ET = mybir.EngineType

    nc = bacc.Bacc(
        "TRN2",
        target_bir_lowering=False,
        debug=False,
        enable_asserts=enable_asserts,
        num_devices=NCORES,
    )
    x_d = nc.dram_tensor("x", [BPC, C, H, W], f32, kind="ExternalInput").ap()
    p_d = nc.dram_tensor("p", [BPC, 3], i32, kind="ExternalInput").ap()
    o_d = nc.dram_tensor("out", [BPC, OC, OHW, OHW], f32, kind="ExternalOutput").ap()

    with ExitStack() as ctx:
        e = ctx.enter_context
        p_sb = e(nc.sbuf_tensor("p_sb", [1, BPC * 3], i32)).ap()
        a_sb = [
            e(nc.sbuf_tensor(f"a_sb{b}", [128, 32 * 64], f32)) for b in range(BPC)
        ]
        v_sb = [
            e(nc.sbuf_tensor(f"v_sb{b}", [128, 2, OHW * OHW], f32))
            for b in range(BPC)
        ]
        s_p = e(nc.semaphore(name="s_p"))
        s_in = [e(nc.semaphore(name=f"s_in{b}")) for b in range(BPC)]
        s_c = [e(nc.semaphore(name=f"s_c{b}")) for b in range(BPC)]
        s_out = e(nc.semaphore(name="s_out"))
        all_sems = [s_p, *s_in, *s_c, s_out]

        a_v = [t.ap().rearrange("p (r c) -> p r c", r=32) for t in a_sb]
        v_v = [t.ap() for t in v_sb]

        def load_p_vals(engine_type):
            _, vals = nc.values_load_multi_w_load_instructions(
                p_sb[0:1, :],
                engines=[engine_type],
                min_val=0,
                max_val=1,
                skip_runtime_bounds_check=True,
            )
            return vals  # (oh0, ow0, r0, oh1, ow1, r1)

        def wait_all_sems(eng):
            # the race validator requires every engine to observe every
            # semaphore's final value before the end-of-kernel clear
            eng.wait_ge(s_p, 16)
            for b in range(BPC):
                eng.wait_ge(s_in[b], 32)
                eng.wait_ge(s_c[b], 2)
            eng.wait_ge(s_out, 16 * BPC)

        block = e(nc.Block(no_gpsimd_drain=no_gpsimd_drain))

        @block.sync
        def _(sync):
            # p -> SBUF, then registers
            sync.dma_start(p_sb[:], p_d.rearrange("b k -> (b k)").unsqueeze(0)).then_inc(
                s_p, 16
            )
            sync.wait_ge(s_p, 16)
            vals = load_p_vals(ET.SP)
            # input DMAs, first halves (rows 0..15 of the 32 needed rows)
            for b in range(BPC):
                oh, r = vals[3 * b + 0], vals[3 * b + 2]
                sync.dma_start(
                    a_v[b][:, 0:16, :],
                    x_d[b][ds(r, 128, 2), ds(oh, 16, 2), :],
                    single_packet=single_packet,
                ).then_inc(s_in[b], 16)
            # output DMAs (dynamic select between the two variants)
            for b in range(BPC):
                r = vals[3 * b + 2]
                sync.wait_ge(s_c[b], 2)
                sync.dma_start(
                    o_d[b].rearrange("c h w -> c (h w)").unsqueeze(1),
                    v_v[b][:, ds(r, 1), :],
                ).then_inc(s_out, 16)
            wait_all_sems(sync)
            sync.drain()

        @block.scalar
        def _(scalar):
            scalar.wait_ge(s_p, 16)
            vals = load_p_vals(ET.Activation)
            # input DMAs, second halves (rows 16..31)
            for b in range(BPC):
                oh, r = vals[3 * b + 0], vals[3 * b + 2]
                scalar.dma_start(
                    a_v[b][:, 16:32, :],
                    x_d[b][ds(r, 128, 2), ds(oh + 32, 16, 2), :],
                    single_packet=single_packet,
                ).then_inc(s_in[b], 16)
            # r=1 variant, columns c=0..15:
            #   c=0:     V1[a, 0]  = A[row 0,  ow + 2a]
            #   c=1..15: V1[a, c]  = A[row 32-c, ow + 2a]  (rows 31..17)
            for b in range(BPC):
                ow = vals[3 * b + 1]
                scalar.wait_ge(s_in[b], 32)
                v1 = v_v[b][:, 1, :].rearrange("p (a c) -> p a c", a=OHW)
                scalar.copy(
                    v1[:, :, 0:1],
                    a_v[b][:, 0:1, ds(ow, 32, 2)].transpose([0, 2, 1]),
                )
                scalar.copy(
                    v1[:, :, 1:16],
                    a_v[b][:, 31:16:-1, ds(ow, 32, 2)].transpose([0, 2, 1]),
                ).then_inc(s_c[b], 1)
            wait_all_sems(scalar)
            scalar.drain()

        @block.vector
        def _(vector):
            vector.wait_ge(s_p, 16)
            vals = load_p_vals(ET.DVE)
            for b in range(BPC):
                ow = vals[3 * b + 1]
                vector.wait_ge(s_in[b], 32)
                # r=0 variant: V0[a, c] = A[a, ow + 2c]
                v0 = v_v[b][:, 0, :].rearrange("p (a c) -> p a c", a=OHW)
                vector.tensor_copy(v0[:], a_v[b][:, :, ds(ow, 32, 2)])
                # r=1 variant, columns c=16..31 (rows 16..1)
                v1 = v_v[b][:, 1, :].rearrange("p (a c) -> p a c", a=OHW)
                vector.tensor_copy(
                    v1[:, :, 16:32],
                    a_v[b][:, 16:0:-1, ds(ow, 32, 2)].transpose([0, 2, 1]),
                ).then_inc(s_c[b], 1)
            wait_all_sems(vector)
            vector.drain()

        @block.tensor
        def _(tensor):
            wait_all_sems(tensor)

        @block.gpsimd
        def _(gpsimd):
            wait_all_sems(gpsimd)
            nums = sorted(s.num for s in all_sems)
            rng = range(nums[0], nums[-1] + 1)
            gpsimd.dma_reset(rng)
            gpsimd.sem_clear(rng)

    nc.compile()
    return nc


def _get_nc():
    if "nc" not in _COMPILED:
        _COMPILED["nc"] = build_nc()
    return _COMPILED["nc"]


def kernel(x: np.ndarray, p: np.ndarray) -> np.ndarray:
    from concourse.bass_utils import run_bass_kernel_spmd

    x = np.ascontiguousarray(x, dtype=np.float32)
    p = np.ascontiguousarray(p, dtype=np.int32)
    assert x.shape == (B, C, H, W) and p.shape == (B, 3)

    nc = _get_nc()
    in_maps = [
        {"x": x[i * BPC : (i + 1) * BPC], "p": p[i * BPC : (i + 1) * BPC]}
        for i in range(NCORES)
    ]
    res = run_bass_kernel_spmd(nc, in_maps, core_ids=list(range(NCORES)))
    return np.concatenate(
        [res.results[i]["out"] for i in range(NCORES)], axis=0
    )
